# revision 32
# baseline (speedup 1.0000x reference)
"""Trainium2 Bass kernel for nn_BiEvidenceNet.

Model (B=1024, R=512, D=256):
    width  = clip(exp(log_width), 1e-3, 50)                  (R,D)
    t_low  = center - width/2 ; t_high = center + width/2    (R,D)
    kappa  = clip(exp(log_kappa), 0.5, 50)                   scalar
    low    = sigmoid(kappa*(t_low - x))   high = sigmoid(kappa*(x - t_high))
    evidence[b,r] = sum_d m*(el*(2*low-1) + eh*(2*high-1))   m=sig(mask), el/eh=tanh(e_*)
    z = sigmoid(6*(evidence - t));  y = z @ head_w.T + head_b

Key identity: 2*sigmoid(u)-1 = tanh(u/2).  When t_low / t_high are constant
across the rule axis (checked at runtime), the (B,R,D) broadcast collapses to
two matmuls over precombined parameter matrices:
    T_s[b,d]    = tanh(khalf*x[b,d] + bias_s[d]),  s in {lo, hi}, khalf=kappa/2
    evidence    = T_lo @ A.T + T_hi @ Bp.T,  A = -(m*el), Bp = m*eh
(The sigmoid/tanh parameter transforms are pure weight preprocessing, folded
on the host; all x-dependent compute stays on device.)

Device dataflow is fully transposed vs the naive layout: D lives on SBUF
partitions (2 k-tiles), evidence accumulates with RULES on PSUM partitions
and batch on the free axis.  That makes -t and the head fold away:
    z = sigmoid(6*ev + (-6t))    ... -6t is a per-partition activation bias
    y[1,b] = sum_rh w_rh^T @ z_rh ... two rank-reduced matmuls into a (1,B2)
PSUM row, so the output leaves as one contiguous DMA row.  Everything is
bf16 on the wire and in the matmuls (fp32 PSUM accumulation); measured
rel err ~3e-3 vs the fp32 reference (gate is 2e-2).

Sharding: 4 batch shards x 2 rule shards over the 8 cores; rule-sharded
partial y rows (each with head_b/2 baked in) are summed on the host.

Toolchain constraint: this walrus encodes at most ONE sync wait per
instruction.  The schedule is shaped so every op has a single-semaphore
dependency: ACT observes each xs DMA once (the tanh_k ops), PE observes
each ab DMA once (1x1 dummy matmuls), and every later cross-engine edge
rides a single already-ticking semaphore.  PE/ACT program order is pinned
with add_dep_helper so the coverage stays valid.

Latency notes (measured, ~1.35 ticks/ns): the NEFF carries ~5.3us of fixed
compiler entry (runtime barriers + TENSOR_LOADs) and ~5.2us of fixed exit
(rendezvous + a 253-semaphore sweep + final barrier) -- a trivial 2-DMA
kernel measures 14.1us.  Each HBM->SBUF DMA pays ~1.0us trigger+receipt
latency regardless of size.  The Tile exit drain is skipped entirely
(TRIM_TAIL): all compute is retired by per-engine program order, and the
1 KB output DMA lands ~4us before the sweep finishes.  Hoisting the input
DMA triggers ahead of the bass-init barrier was tried and REGRESSED ~1us
(the barrier's InstDrain waits out in-flight HWDGE emission).
"""

import numpy as np
import ml_dtypes

B, R, D = 1024, 512, 256
N_CORES = 8
NB = 4                      # batch shards
NR = 2                      # rule shards
B2 = B // NB                # batch rows per core (256)
R2 = R // NR                # rules per core (256)
KT = D // 128               # contraction k-tiles
BETA = 6.0
TRIM_TAIL = True            # skip Tile's sem-clear + second barrier (one-shot NEFF)

_F32 = np.float32
_BF16 = ml_dtypes.bfloat16

# xs column map (bf16): [x_k0 | blo_k0 bhi_k0 blo_k1 bhi_k1 zb0 zb1 | x_k1]
# (bias/zb columns ride with the k0 slab so tanh_k0 can start off DMA #1
# while the k1 slab is still streaming in DMA #2)
XB = 2 * KT + NR                    # 6 bias columns
XC = KT * B2 + XB                   # 518
X1 = B2 + XB                        # start of the k1 slab / end of DMA #1
# ab column map (bf16): [A_k0 | Bp_k0 | A_k1 | Bp_k1 | w0 w1]
ABC = 2 * KT * R2 + NR              # 1026


def _single_wait_tile_context(nc, tile):
    """TileContext whose tail carries at most one sync wait per instruction.

    DMA-lane procs are skipped in the exit drain: every input DMA is already
    observed by its compute consumer, and the only unobserved tick is the
    1 KB output DMA, which lands within the first ~1.5us of the compiler's
    ~5us semaphore-sweep epilogue.
    """
    from concourse.vector_clock import ScopedClock, VectorClock
    from concourse.tile_sem_assignment import PROC_NAME_TO_IDX

    dma_procs = {v for k, v in PROC_NAME_TO_IDX.items() if k.startswith("DMA")}

    class SingleWaitTileContext(tile.TileContext):
        def _drain_and_barrier(self, tick_clock, wait_clock):
            if TRIM_TAIL:
                # One-shot NEFF: each engine's own program order already
                # retires its work, every input DMA was observed by its
                # consumer, and the compiler's exit rendezvous + ~5us
                # semaphore sweep follow immediately -- ample cover for the
                # 1 KB output DMA still in flight.  So no NOP waits, no
                # drain, no extra barrier here.
                assert self.sems is not None
                popped = self.nc._tile_sem_poison_stack.pop()
                assert popped is self._sem_poison
                return
            gc = tick_clock.global_clock
            n = len(gc)
            for proc in range(n):
                if gc[proc] <= 0 or proc in dma_procs:
                    continue
                vec = VectorClock([gc[i] if i == proc else 0 for i in range(n)])
                inst = self.nc.sync.nop(nofuse=True)
                wait_clock.add_sem_waits(inst.ins, ScopedClock({None: vec}))
            # the NOP chain above already waited out every proc, so the drain
            # itself needs no waits (walrus would reject a multi-wait drain)
            self.nc.sync.drain()
            self.nc.all_engine_barrier()
            assert self.sems is not None
            popped = self.nc._tile_sem_poison_stack.pop()
            assert popped is self._sem_poison
            self.nc.clear_and_free_semaphores(
                list(self.sems.allocated().values()))
            self.nc.all_engine_barrier()

    return SingleWaitTileContext(nc)


def _build_nc(khalf: float, head_b_half: float):
    import concourse.bass as bass
    import concourse.mybir as mybir
    from concourse import tile
    from concourse.tile_rust import add_dep_helper

    f32 = mybir.dt.float32
    bf16 = mybir.dt.bfloat16
    AF = mybir.ActivationFunctionType

    nc = bass.Bass()
    d_xs = nc.declare_dram_parameter("xs", [128, XC], bf16, isOutput=False)
    d_ab = nc.declare_dram_parameter("ab", [128, ABC], bf16, isOutput=False)
    d_y = nc.declare_dram_parameter("y", [1, B2], f32, isOutput=True)

    with _single_wait_tile_context(nc, tile) as tc:
        with (
            tc.tile_pool(name="sb", bufs=1) as sb,
            tc.tile_pool(name="ps", bufs=1, space="PSUM") as ps,
        ):
            xs = sb.tile([128, XC], bf16, tag="xs")
            ab = sb.tile([128, ABC], bf16, tag="ab")
            t4 = sb.tile([128, 2 * KT, B2], bf16, tag="t4")
            z = sb.tile([128, NR, B2], bf16, tag="z")
            yrow = sb.tile([1, B2], f32, tag="yrow")

            ev0 = ps.tile([128, B2], f32, tag="ev0")
            ev1 = ps.tile([128, B2], f32, tag="ev1")
            ev = [ev0, ev1]
            yp = ps.tile([1, B2], f32, tag="yp")
            scratch_ps = ps.tile([1, 1], f32, tag="scratch_ps")

            # DMA plan: the k0 half of ab (slabs 0-1) rides the ACT HWDGE
            # ring as ACT's first op (the activation-table load fills the
            # rest of the trigger window); xs is split at the k0/k1 boundary
            # on the sync ring so tanh_k0 starts one stream earlier, and the
            # k1 half of ab (+ head w) queues third on the sync ring --
            # everything it gates happens after tanh_k1 anyway.
            AB1 = 2 * R2                      # ab column split point
            nc.scalar.dma_start(ab[:, 0:AB1], d_ab[:, 0:AB1])
            nc.sync.dma_start(xs[:, 0:X1], d_xs[:, 0:X1])
            nc.sync.dma_start(xs[:, X1:XC], d_xs[:, X1:XC])
            nc.sync.dma_start(ab[:, AB1:ABC], d_ab[:, AB1:ABC])

            # T_s = tanh(khalf*x + bias_s): 4 ACT ops; tanh_k0 waits DMA
            # xs0, tanh_k1 waits xs1, program order covers the rest.
            prev_act = None
            for i, (k, s) in enumerate([(k, s) for k in range(KT)
                                        for s in range(2)]):
                xcol = 0 if k == 0 else X1
                a = nc.scalar.activation(
                    t4[:, i, :], xs[:, xcol:xcol + B2], AF.Tanh,
                    bias=xs[:, B2 + 2 * k + s:B2 + 2 * k + s + 1],
                    scale=khalf,
                )
                if prev_act is not None:
                    add_dep_helper(a.ins, prev_act.ins, sync=False,
                                   reason="ACT program order")
                prev_act = a

            # PE observes each ab DMA exactly once via a 1x1 dummy matmul;
            # every data matmul then carries only its ACT (tanh) wait.
            # Pairs are k-major -- the rh1 matmul of slab i fills the gap
            # while PE waits for tanh_{i+1}, so ev1 closes ~200 ticks after
            # ev0 instead of 4 matmuls later.
            prev_pe = nc.tensor.matmul(scratch_ps[:], ab[0:1, 0:1],
                                       ab[0:1, 0:1], start=True, stop=True)
            # k-major pairs for slabs 0-1 (rh1 mms fill the tanh-wait
            # gaps); for slabs 2-3, both rh0 mms go FIRST so ev0's stop
            # lands right after tanh3 and z0 isn't stuck behind the rh1
            # stragglers (~0.2us earlier close).
            order = [(0, 0), (0, 1), (1, 0), (1, 1),
                     (2, 0), (3, 0), (2, 1), (3, 1)]
            for i, rh in order:
                if (i, rh) == (2, 0):  # slabs 2-3 + w ride the second ab DMA
                    mm = nc.tensor.matmul(
                        scratch_ps[:], ab[0:1, AB1:AB1 + 1],
                        ab[0:1, AB1:AB1 + 1], start=True, stop=True)
                    add_dep_helper(mm.ins, prev_pe.ins, sync=False,
                                   reason="PE program order (single-wait)")
                    prev_pe = mm
                mm = nc.tensor.matmul(
                    ev[rh][:],
                    ab[:, i * R2 + rh * 128:i * R2 + (rh + 1) * 128],
                    t4[:, i, :],
                    start=(i == 0), stop=(i == 2 * KT - 1))
                add_dep_helper(mm.ins, prev_pe.ins, sync=False,
                               reason="PE program order (single-wait)")
                prev_pe = mm

            # z_rh = sigmoid(6*ev + (-6 t)) -- bias column rides in xs, which
            # ACT has already observed.
            zops = []
            for rh in range(NR):
                zc = B2 + 2 * KT + rh
                a = nc.scalar.activation(z[:, rh, :], ev[rh][:], AF.Sigmoid,
                                         bias=xs[:, zc:zc + 1], scale=BETA)
                add_dep_helper(a.ins, prev_act.ins, sync=False,
                               reason="ACT program order")
                prev_act = a
                zops.append(a)

            # head: y[1,b] accumulates w_rh^T @ z_rh over the two rule halves
            for rh in range(NR):
                wc = 2 * KT * R2 + rh
                mm = nc.tensor.matmul(yp[:], ab[:, wc:wc + 1], z[:, rh, :],
                                      start=(rh == 0), stop=(rh == NR - 1))
                add_dep_helper(mm.ins, prev_pe.ins, sync=False,
                               reason="PE program order (single-wait)")
                prev_pe = mm

            # + head_b/2, PSUM -> SBUF, and the row out -- ALL on ACT: the
            # copy (AF.Copy takes a float immediate bias; Identity would
            # pull in a const AP whose memset is no longer barrier-ordered)
            # carries the single PE wait, and the trigger follows by program
            # order with no wait at all.  This removes both cross-engine
            # hops from the tail and makes ACT the last exit-rendezvous
            # arrival ~0.4us earlier than sync-after-DVE was.  (gpsimd/SWDGE
            # and DVE+ACT-split tails both REGRESSED ~2us.)
            a = nc.scalar.activation(yrow[:], yp[:], AF.Copy,
                                     bias=head_b_half)
            add_dep_helper(a.ins, prev_act.ins, sync=False,
                           reason="ACT program order")
            nc.scalar.dma_start(d_y[:], yrow[:])

    # Drop the bass-init all-engine barrier (5 InstDrain + 6
    # InstEventSemaphore in the entry block).  It only orders the const-AP
    # memsets against readers, and this program references no const APs
    # (all activation biases are AP columns).  The compiler wrapper's own
    # entry barrier already synchronized the engines, so each engine can
    # fall straight through its preamble into the body -- the DMA triggers
    # fire ~0.4us earlier.  (Hoisting triggers ABOVE the barrier instead
    # was tried and REGRESSED ~1us: the barrier's per-engine InstDrain
    # waits out in-flight HWDGE emission.)
    main_blk = nc.m.functions[0].blocks[0]
    main_blk.instructions[:] = [
        i for i in main_blk.instructions
        if type(i).__name__ not in ("InstDrain", "InstEventSemaphore")
    ]
    nc.finalize()
    return nc


def _fast_path_inputs(x, a_mat, b_mat, tau_lo, tau_hi, khalf, t, head_w,
                      head_b):
    """Per-core input maps.  Host work: parameter folding + transposes."""
    xT = np.ascontiguousarray(x.T, dtype=_F32)            # (D, B)
    aT = np.ascontiguousarray(a_mat.T, dtype=_F32)        # (D, R)
    bT = np.ascontiguousarray(b_mat.T, dtype=_F32)        # (D, R)
    blo = (-_F32(khalf) * tau_lo).astype(_F32)            # (D,)
    bhi = (-_F32(khalf) * tau_hi).astype(_F32)

    xss = []
    for i in range(NB):
        xs = np.zeros((128, XC), dtype=_F32)
        xs[:, 0:B2] = xT[0:128, i * B2:(i + 1) * B2]
        xs[:, X1:X1 + B2] = xT[128:256, i * B2:(i + 1) * B2]
        for k in range(KT):
            xs[:, B2 + 2 * k] = blo[k * 128:(k + 1) * 128]
            xs[:, B2 + 2 * k + 1] = bhi[k * 128:(k + 1) * 128]
        xss.append(xs)

    abss = []
    zbs = []
    for j in range(NR):
        rs = slice(j * R2, (j + 1) * R2)
        abm = np.zeros((128, ABC), dtype=_F32)
        for k in range(KT):
            abm[:, (2 * k) * R2:(2 * k + 1) * R2] = aT[k * 128:(k + 1) * 128, rs]
            abm[:, (2 * k + 1) * R2:(2 * k + 2) * R2] = bT[k * 128:(k + 1) * 128, rs]
        w = head_w.reshape(R).astype(_F32)[rs]
        for rh in range(NR):
            abm[:, 2 * KT * R2 + rh] = w[rh * 128:(rh + 1) * 128]
        abss.append(abm)
        zb = np.zeros((128, NR), dtype=_F32)
        for rh in range(NR):
            zb[:, rh] = -_F32(BETA) * t[rs][rh * 128:(rh + 1) * 128]
        zbs.append(zb)

    in_maps = []
    for c in range(N_CORES):
        i, j = c % NB, c // NB
        xs = xss[i].copy()
        xs[:, B2 + 2 * KT:B2 + 2 * KT + NR] = zbs[j]
        in_maps.append({
            "xs": xs.astype(_BF16),
            "ab": abss[j].astype(_BF16),
        })
    return in_maps


def _reference_numpy(x, center, log_width, e_low, e_high, mask, log_kappa, t,
                     head_w, head_b):
    """General fallback, exact reference semantics in fp32 numpy (chunked)."""
    width = np.clip(np.exp(log_width, dtype=_F32), 1e-3, 50.0).astype(_F32)
    t_low = (center - _F32(0.5) * width).astype(_F32)
    t_high = (center + _F32(0.5) * width).astype(_F32)
    kappa = np.clip(np.exp(_F32(log_kappa)), 0.5, 50.0).astype(_F32)

    def sig(v):
        return _F32(0.5) * (np.tanh(_F32(0.5) * v) + _F32(1.0))

    m = sig(mask.astype(_F32))
    el = np.tanh(e_low.astype(_F32))
    eh = np.tanh(e_high.astype(_F32))
    out = np.empty(x.shape[0], dtype=_F32)
    for s in range(0, x.shape[0], 64):
        xc = x[s:s + 64].astype(_F32)
        low = sig(kappa * (t_low[None] - xc[:, None, :]))
        high = sig(kappa * (xc[:, None, :] - t_high[None]))
        evidence = np.sum(
            m[None] * (el[None] * (2 * low - 1) + eh[None] * (2 * high - 1)),
            axis=2, dtype=_F32)
        z = sig(_F32(BETA) * (evidence - t[None].astype(_F32)))
        out[s:s + 64] = z @ head_w.reshape(-1).astype(_F32) + _F32(head_b)
    return out


def kernel_with_stats(trace=False, **inputs):
    x = np.asarray(inputs["x"], dtype=_F32)
    center = np.asarray(inputs["center"], dtype=_F32)
    log_width = np.asarray(inputs["log_width"], dtype=_F32)
    e_low = np.asarray(inputs["e_low"], dtype=_F32)
    e_high = np.asarray(inputs["e_high"], dtype=_F32)
    mask = np.asarray(inputs["mask"], dtype=_F32)
    log_kappa = np.asarray(inputs["log_kappa"], dtype=_F32)
    t = np.asarray(inputs["t"], dtype=_F32)
    head_w = np.asarray(inputs["head_w"], dtype=_F32)
    head_b = np.asarray(inputs["head_b"], dtype=_F32)

    assert x.shape == (B, D) and mask.shape == (R, D)

    # fast-path structural check: thresholds constant across the rule axis
    width = np.clip(np.exp(log_width), 1e-3, 50.0).astype(_F32)
    t_low = (center - _F32(0.5) * width).astype(_F32)
    t_high = (center + _F32(0.5) * width).astype(_F32)
    if not (np.all(t_low == t_low[0:1]) and np.all(t_high == t_high[0:1])):
        out = _reference_numpy(x, center, log_width, e_low, e_high, mask,
                               log_kappa, t, head_w, head_b)
        return out, None

    from concourse.bass_utils import run_bass_kernel_spmd

    kappa = np.clip(np.exp(_F32(log_kappa)), 0.5, 50.0).astype(_F32)
    khalf = float(kappa) / 2.0

    def sig(v):
        return _F32(0.5) * (np.tanh(_F32(0.5) * v) + _F32(1.0))

    a_mat = (-sig(mask) * np.tanh(e_low)).astype(_F32)     # (R, D)
    b_mat = (sig(mask) * np.tanh(e_high)).astype(_F32)

    in_maps = _fast_path_inputs(x, a_mat, b_mat, t_low[0], t_high[0], khalf,
                                t, head_w, head_b)

    nc = _build_nc(khalf, float(head_b.reshape(-1)[0]) / 2.0)
    res = run_bass_kernel_spmd(nc, in_maps, list(range(N_CORES)), trace=trace)
    out = np.zeros(B, dtype=np.float64)
    for c in range(N_CORES):
        i = c % NB
        out[i * B2:(i + 1) * B2] += res.results[c]["y"].reshape(B2).astype(np.float64)
    return out.astype(_F32), res


def kernel(**inputs):
    out, _ = kernel_with_stats(**inputs)
    return out


# revision 33
# speedup vs baseline: 1.0259x; 1.0259x over previous
"""Trainium2 Bass kernel for nn_BiEvidenceNet.

Model (B=1024, R=512, D=256):
    width  = clip(exp(log_width), 1e-3, 50)                  (R,D)
    t_low  = center - width/2 ; t_high = center + width/2    (R,D)
    kappa  = clip(exp(log_kappa), 0.5, 50)                   scalar
    low    = sigmoid(kappa*(t_low - x))   high = sigmoid(kappa*(x - t_high))
    evidence[b,r] = sum_d m*(el*(2*low-1) + eh*(2*high-1))   m=sig(mask), el/eh=tanh(e_*)
    z = sigmoid(6*(evidence - t));  y = z @ head_w.T + head_b

Key identity: 2*sigmoid(u)-1 = tanh(u/2).  When t_low / t_high are constant
across the rule axis (checked at runtime), the (B,R,D) broadcast collapses to
two matmuls over precombined parameter matrices:
    T_s[b,d]    = tanh(khalf*x[b,d] + bias_s[d]),  s in {lo, hi}, khalf=kappa/2
    evidence    = T_lo @ A.T + T_hi @ Bp.T,  A = -(m*el), Bp = m*eh
(The sigmoid/tanh parameter transforms are pure weight preprocessing, folded
on the host; all x-dependent compute stays on device.)

Device dataflow is fully transposed vs the naive layout: D lives on SBUF
partitions (2 k-tiles), evidence accumulates with RULES on PSUM partitions
and batch on the free axis.  That makes -t and the head fold away:
    z = sigmoid(6*ev + (-6t))    ... -6t is a per-partition activation bias
    y[1,b] = sum_rh w_rh^T @ z_rh ... two rank-reduced matmuls into a (1,B2)
PSUM row, so the output leaves as one contiguous DMA row.  Everything is
bf16 on the wire and in the matmuls (fp32 PSUM accumulation); measured
rel err ~3e-3 vs the fp32 reference (gate is 2e-2).

Sharding: 4 batch shards x 2 rule shards over the 8 cores; rule-sharded
partial y rows (each with head_b/2 baked in) are summed on the host.

Toolchain constraint: this walrus encodes at most ONE sync wait per
instruction.  The schedule is shaped so every op has a single-semaphore
dependency: ACT observes each xs DMA once (the tanh_k ops), PE observes
each ab DMA once (1x1 dummy matmuls), and every later cross-engine edge
rides a single already-ticking semaphore.  PE/ACT program order is pinned
with add_dep_helper so the coverage stays valid.

Latency notes (measured, ~1.35 ticks/ns): the NEFF carries ~5.3us of fixed
compiler entry (runtime barriers + TENSOR_LOADs) and ~5.2us of fixed exit
(rendezvous + a 253-semaphore sweep + final barrier) -- a trivial 2-DMA
kernel measures 14.1us.  Each HBM->SBUF DMA pays ~1.0us trigger+receipt
latency regardless of size.  The Tile exit drain is skipped entirely
(TRIM_TAIL): all compute is retired by per-engine program order, and the
1 KB output DMA lands ~4us before the sweep finishes.  Hoisting the input
DMA triggers ahead of the bass-init barrier was tried and REGRESSED ~1us
(the barrier's InstDrain waits out in-flight HWDGE emission).
"""

import numpy as np
import ml_dtypes

B, R, D = 1024, 512, 256
N_CORES = 8
NB = 4                      # batch shards
NR = 2                      # rule shards
B2 = B // NB                # batch rows per core (256)
R2 = R // NR                # rules per core (256)
KT = D // 128               # contraction k-tiles
BETA = 6.0
TRIM_TAIL = True            # skip Tile's sem-clear + second barrier (one-shot NEFF)

_F32 = np.float32
_BF16 = ml_dtypes.bfloat16

# xs column map (bf16): [x_k0 | blo_k0 bhi_k0 blo_k1 bhi_k1 zb0 zb1 | x_k1]
# (bias/zb columns ride with the k0 slab so tanh_k0 can start off DMA #1
# while the k1 slab is still streaming in DMA #2)
XB = 2 * KT + NR                    # 6 bias columns
XC = KT * B2 + XB                   # 518
X1 = B2 + XB                        # start of the k1 slab / end of DMA #1
# ab column map (bf16): [A_k0 | Bp_k0 | A_k1 | Bp_k1 | w0 w1]
ABC = 2 * KT * R2 + NR              # 1026


def _single_wait_tile_context(nc, tile):
    """TileContext whose tail carries at most one sync wait per instruction.

    DMA-lane procs are skipped in the exit drain: every input DMA is already
    observed by its compute consumer, and the only unobserved tick is the
    1 KB output DMA, which lands within the first ~1.5us of the compiler's
    ~5us semaphore-sweep epilogue.
    """
    from concourse.vector_clock import ScopedClock, VectorClock
    from concourse.tile_sem_assignment import PROC_NAME_TO_IDX

    dma_procs = {v for k, v in PROC_NAME_TO_IDX.items() if k.startswith("DMA")}

    class SingleWaitTileContext(tile.TileContext):
        def _drain_and_barrier(self, tick_clock, wait_clock):
            if TRIM_TAIL:
                # One-shot NEFF: each engine's own program order already
                # retires its work, every input DMA was observed by its
                # consumer, and the compiler's exit rendezvous + ~5us
                # semaphore sweep follow immediately -- ample cover for the
                # 1 KB output DMA still in flight.  So no NOP waits, no
                # drain, no extra barrier here.
                assert self.sems is not None
                popped = self.nc._tile_sem_poison_stack.pop()
                assert popped is self._sem_poison
                return
            gc = tick_clock.global_clock
            n = len(gc)
            for proc in range(n):
                if gc[proc] <= 0 or proc in dma_procs:
                    continue
                vec = VectorClock([gc[i] if i == proc else 0 for i in range(n)])
                inst = self.nc.sync.nop(nofuse=True)
                wait_clock.add_sem_waits(inst.ins, ScopedClock({None: vec}))
            # the NOP chain above already waited out every proc, so the drain
            # itself needs no waits (walrus would reject a multi-wait drain)
            self.nc.sync.drain()
            self.nc.all_engine_barrier()
            assert self.sems is not None
            popped = self.nc._tile_sem_poison_stack.pop()
            assert popped is self._sem_poison
            self.nc.clear_and_free_semaphores(
                list(self.sems.allocated().values()))
            self.nc.all_engine_barrier()

    return SingleWaitTileContext(nc)


def _build_nc(khalf: float, head_b_half: float):
    import concourse.bass as bass
    import concourse.mybir as mybir
    from concourse import tile
    from concourse.tile_rust import add_dep_helper

    f32 = mybir.dt.float32
    bf16 = mybir.dt.bfloat16
    AF = mybir.ActivationFunctionType

    nc = bass.Bass()
    d_xs = nc.declare_dram_parameter("xs", [128, XC], bf16, isOutput=False)
    d_ab = nc.declare_dram_parameter("ab", [128, ABC], bf16, isOutput=False)
    d_y = nc.declare_dram_parameter("y", [1, B2], f32, isOutput=True)

    with _single_wait_tile_context(nc, tile) as tc:
        with (
            tc.tile_pool(name="sb", bufs=1) as sb,
            tc.tile_pool(name="ps", bufs=1, space="PSUM") as ps,
        ):
            xs = sb.tile([128, XC], bf16, tag="xs")
            ab = sb.tile([128, ABC], bf16, tag="ab")
            t4 = sb.tile([128, 2 * KT, B2], bf16, tag="t4")
            z = sb.tile([128, NR, B2], bf16, tag="z")
            yrow = sb.tile([1, B2], f32, tag="yrow")

            ev0 = ps.tile([128, B2], f32, tag="ev0")
            ev1 = ps.tile([128, B2], f32, tag="ev1")
            ev = [ev0, ev1]
            yp = ps.tile([1, B2], f32, tag="yp")
            scratch_ps = ps.tile([1, 1], f32, tag="scratch_ps")

            # DMA plan: the k0 half of ab (slabs 0-1) rides the ACT HWDGE
            # ring as ACT's first op (the activation-table load fills the
            # rest of the trigger window); xs is split at the k0/k1 boundary
            # on the sync ring so tanh_k0 starts one stream earlier, and the
            # k1 half of ab (+ head w) queues third on the sync ring --
            # everything it gates happens after tanh_k1 anyway.
            AB1 = 2 * R2                      # ab column split point
            nc.scalar.dma_start(ab[:, 0:AB1], d_ab[:, 0:AB1])
            nc.sync.dma_start(xs[:, 0:X1], d_xs[:, 0:X1])
            nc.sync.dma_start(xs[:, X1:XC], d_xs[:, X1:XC])
            nc.sync.dma_start(ab[:, AB1:ABC], d_ab[:, AB1:ABC])

            # T_s = tanh(khalf*x + bias_s): 4 ACT ops; tanh_k0 waits DMA
            # xs0, tanh_k1 waits xs1, program order covers the rest.
            prev_act = None
            for i, (k, s) in enumerate([(k, s) for k in range(KT)
                                        for s in range(2)]):
                xcol = 0 if k == 0 else X1
                a = nc.scalar.activation(
                    t4[:, i, :], xs[:, xcol:xcol + B2], AF.Tanh,
                    bias=xs[:, B2 + 2 * k + s:B2 + 2 * k + s + 1],
                    scale=khalf,
                )
                if prev_act is not None:
                    add_dep_helper(a.ins, prev_act.ins, sync=False,
                                   reason="ACT program order")
                prev_act = a

            # PE observes each ab DMA exactly once via a 1x1 dummy matmul;
            # every data matmul then carries only its ACT (tanh) wait.
            # Pairs are k-major -- the rh1 matmul of slab i fills the gap
            # while PE waits for tanh_{i+1}, so ev1 closes ~200 ticks after
            # ev0 instead of 4 matmuls later.
            prev_pe = nc.tensor.matmul(scratch_ps[:], ab[0:1, 0:1],
                                       ab[0:1, 0:1], start=True, stop=True)
            # k-major pairs for slabs 0-1 (rh1 mms fill the tanh-wait
            # gaps); for slabs 2-3, both rh0 mms go FIRST so ev0's stop
            # lands right after tanh3 and z0 isn't stuck behind the rh1
            # stragglers (~0.2us earlier close).
            order = [(0, 0), (0, 1), (1, 0), (1, 1),
                     (2, 0), (3, 0), (2, 1), (3, 1)]
            for i, rh in order:
                if (i, rh) == (2, 0):  # slabs 2-3 + w ride the second ab DMA
                    mm = nc.tensor.matmul(
                        scratch_ps[:], ab[0:1, AB1:AB1 + 1],
                        ab[0:1, AB1:AB1 + 1], start=True, stop=True)
                    add_dep_helper(mm.ins, prev_pe.ins, sync=False,
                                   reason="PE program order (single-wait)")
                    prev_pe = mm
                mm = nc.tensor.matmul(
                    ev[rh][:],
                    ab[:, i * R2 + rh * 128:i * R2 + (rh + 1) * 128],
                    t4[:, i, :],
                    start=(i == 0), stop=(i == 2 * KT - 1))
                add_dep_helper(mm.ins, prev_pe.ins, sync=False,
                               reason="PE program order (single-wait)")
                prev_pe = mm

            # z_rh = sigmoid(6*ev + (-6 t)) -- bias column rides in xs, which
            # ACT has already observed.
            zops = []
            for rh in range(NR):
                zc = B2 + 2 * KT + rh
                a = nc.scalar.activation(z[:, rh, :], ev[rh][:], AF.Sigmoid,
                                         bias=xs[:, zc:zc + 1], scale=BETA)
                add_dep_helper(a.ins, prev_act.ins, sync=False,
                               reason="ACT program order")
                prev_act = a
                zops.append(a)

            # head: y[1,b] accumulates w_rh^T @ z_rh over the two rule halves
            for rh in range(NR):
                wc = 2 * KT * R2 + rh
                mm = nc.tensor.matmul(yp[:], ab[:, wc:wc + 1], z[:, rh, :],
                                      start=(rh == 0), stop=(rh == NR - 1))
                add_dep_helper(mm.ins, prev_pe.ins, sync=False,
                               reason="PE program order (single-wait)")
                prev_pe = mm

            # + head_b/2, PSUM -> SBUF, and the row out -- ALL on ACT: the
            # copy (AF.Copy takes a float immediate bias; Identity would
            # pull in a const AP whose memset is no longer barrier-ordered)
            # carries the single PE wait, and the trigger follows by program
            # order with no wait at all.  This removes both cross-engine
            # hops from the tail and makes ACT the last exit-rendezvous
            # arrival ~0.4us earlier than sync-after-DVE was.  (gpsimd/SWDGE
            # and DVE+ACT-split tails both REGRESSED ~2us.)
            a = nc.scalar.activation(yrow[:], yp[:], AF.Copy,
                                     bias=head_b_half)
            add_dep_helper(a.ins, prev_act.ins, sync=False,
                           reason="ACT program order")
            nc.scalar.dma_start(d_y[:], yrow[:])

    # Drop the bass-init all-engine barrier (5 InstDrain + 6
    # InstEventSemaphore in the entry block).  It only orders the const-AP
    # memsets against readers, and this program references no const APs
    # (all activation biases are AP columns).  The compiler wrapper's own
    # entry barrier already synchronized the engines, so each engine can
    # fall straight through its preamble into the body -- the DMA triggers
    # fire ~0.4us earlier.  (Hoisting triggers ABOVE the barrier instead
    # was tried and REGRESSED ~1us: the barrier's per-engine InstDrain
    # waits out in-flight HWDGE emission.)
    main_blk = nc.m.functions[0].blocks[0]
    main_blk.instructions[:] = [
        i for i in main_blk.instructions
        if type(i).__name__ not in ("InstDrain", "InstEventSemaphore")
    ]
    nc.finalize()
    return nc


def _fast_path_inputs(x, a_mat, b_mat, tau_lo, tau_hi, khalf, t, head_w,
                      head_b):
    """Per-core input maps.  Host work: parameter folding + transposes."""
    xT = np.ascontiguousarray(x.T, dtype=_F32)            # (D, B)
    aT = np.ascontiguousarray(a_mat.T, dtype=_F32)        # (D, R)
    bT = np.ascontiguousarray(b_mat.T, dtype=_F32)        # (D, R)
    blo = (-_F32(khalf) * tau_lo).astype(_F32)            # (D,)
    bhi = (-_F32(khalf) * tau_hi).astype(_F32)

    xss = []
    for i in range(NB):
        xs = np.zeros((128, XC), dtype=_F32)
        xs[:, 0:B2] = xT[0:128, i * B2:(i + 1) * B2]
        xs[:, X1:X1 + B2] = xT[128:256, i * B2:(i + 1) * B2]
        for k in range(KT):
            xs[:, B2 + 2 * k] = blo[k * 128:(k + 1) * 128]
            xs[:, B2 + 2 * k + 1] = bhi[k * 128:(k + 1) * 128]
        xss.append(xs)

    abss = []
    zbs = []
    for j in range(NR):
        rs = slice(j * R2, (j + 1) * R2)
        abm = np.zeros((128, ABC), dtype=_F32)
        for k in range(KT):
            abm[:, (2 * k) * R2:(2 * k + 1) * R2] = aT[k * 128:(k + 1) * 128, rs]
            abm[:, (2 * k + 1) * R2:(2 * k + 2) * R2] = bT[k * 128:(k + 1) * 128, rs]
        w = head_w.reshape(R).astype(_F32)[rs]
        for rh in range(NR):
            abm[:, 2 * KT * R2 + rh] = w[rh * 128:(rh + 1) * 128]
        abss.append(abm)
        zb = np.zeros((128, NR), dtype=_F32)
        for rh in range(NR):
            zb[:, rh] = -_F32(BETA) * t[rs][rh * 128:(rh + 1) * 128]
        zbs.append(zb)

    in_maps = []
    for c in range(N_CORES):
        i, j = c % NB, c // NB
        xs = xss[i].copy()
        xs[:, B2 + 2 * KT:B2 + 2 * KT + NR] = zbs[j]
        in_maps.append({
            "xs": xs.astype(_BF16),
            "ab": abss[j].astype(_BF16),
        })
    return in_maps


def _reference_numpy(x, center, log_width, e_low, e_high, mask, log_kappa, t,
                     head_w, head_b):
    """General fallback, exact reference semantics in fp32 numpy (chunked)."""
    width = np.clip(np.exp(log_width, dtype=_F32), 1e-3, 50.0).astype(_F32)
    t_low = (center - _F32(0.5) * width).astype(_F32)
    t_high = (center + _F32(0.5) * width).astype(_F32)
    kappa = np.clip(np.exp(_F32(log_kappa)), 0.5, 50.0).astype(_F32)

    def sig(v):
        return _F32(0.5) * (np.tanh(_F32(0.5) * v) + _F32(1.0))

    m = sig(mask.astype(_F32))
    el = np.tanh(e_low.astype(_F32))
    eh = np.tanh(e_high.astype(_F32))
    out = np.empty(x.shape[0], dtype=_F32)
    for s in range(0, x.shape[0], 64):
        xc = x[s:s + 64].astype(_F32)
        low = sig(kappa * (t_low[None] - xc[:, None, :]))
        high = sig(kappa * (xc[:, None, :] - t_high[None]))
        evidence = np.sum(
            m[None] * (el[None] * (2 * low - 1) + eh[None] * (2 * high - 1)),
            axis=2, dtype=_F32)
        z = sig(_F32(BETA) * (evidence - t[None].astype(_F32)))
        out[s:s + 64] = z @ head_w.reshape(-1).astype(_F32) + _F32(head_b)
    return out


def kernel_with_stats(trace=False, **inputs):
    x = np.asarray(inputs["x"], dtype=_F32)
    center = np.asarray(inputs["center"], dtype=_F32)
    log_width = np.asarray(inputs["log_width"], dtype=_F32)
    e_low = np.asarray(inputs["e_low"], dtype=_F32)
    e_high = np.asarray(inputs["e_high"], dtype=_F32)
    mask = np.asarray(inputs["mask"], dtype=_F32)
    log_kappa = np.asarray(inputs["log_kappa"], dtype=_F32)
    t = np.asarray(inputs["t"], dtype=_F32)
    head_w = np.asarray(inputs["head_w"], dtype=_F32)
    head_b = np.asarray(inputs["head_b"], dtype=_F32)

    assert x.shape == (B, D) and mask.shape == (R, D)

    # fast-path structural check: thresholds constant across the rule axis
    width = np.clip(np.exp(log_width), 1e-3, 50.0).astype(_F32)
    t_low = (center - _F32(0.5) * width).astype(_F32)
    t_high = (center + _F32(0.5) * width).astype(_F32)
    if not (np.all(t_low == t_low[0:1]) and np.all(t_high == t_high[0:1])):
        out = _reference_numpy(x, center, log_width, e_low, e_high, mask,
                               log_kappa, t, head_w, head_b)
        return out, None

    from concourse.bass_utils import run_bass_kernel_spmd

    kappa = np.clip(np.exp(_F32(log_kappa)), 0.5, 50.0).astype(_F32)
    khalf = float(kappa) / 2.0

    def sig(v):
        return _F32(0.5) * (np.tanh(_F32(0.5) * v) + _F32(1.0))

    a_mat = (-sig(mask) * np.tanh(e_low)).astype(_F32)     # (R, D)
    b_mat = (sig(mask) * np.tanh(e_high)).astype(_F32)

    in_maps = _fast_path_inputs(x, a_mat, b_mat, t_low[0], t_high[0], khalf,
                                t, head_w, head_b)

    # head_b is added on the HOST during the gather: the device bias is
    # always 0.0, so the AF.Copy immediate-bias path (untestable with this
    # problem's head_b == 0) is never relied on, and the NEFF is
    # independent of the head_b value.
    nc = _build_nc(khalf, 0.0)
    res = run_bass_kernel_spmd(nc, in_maps, list(range(N_CORES)), trace=trace)
    out = np.full(B, float(head_b.reshape(-1)[0]), dtype=np.float64)
    for c in range(N_CORES):
        i = c % NB
        out[i * B2:(i + 1) * B2] += res.results[c]["y"].reshape(B2).astype(np.float64)
    return out.astype(_F32), res


def kernel(**inputs):
    out, _ = kernel_with_stats(**inputs)
    return out
